# revision 7
# baseline (speedup 1.0000x reference)
"""AFT-Local autoregressive attention on 8 Trainium2 NeuronCores.

Sequence-parallel decomposition: core i owns tokens [256i, 256i+256) (two
globally-aligned 128-token blocks m=2i, 2i+1) and recomputes the previous
128-token block as halo. With ek = exp(k) (the reference's max_logit cancels
in num/den), the AFT mixing for output block m is a banded matmul

    num[block m] = C_m (colsums of blocks <= m-2)  +  Wpair[m] @ [ekv[m-1]; ekv[m]]

where Wpair[tr, tc2] = 1 for tc2 <= tr+96, exp(pb) on the 32-band, 0 on the
future. C_m comes from one 8-core AllGather of per-block column sums
([4,1024] fp32 per core), folded in as a K=32 matmul. All matmuls run in
float32r (1 cycle/row).
"""
import sys
sys.path.insert(0, "/opt/trn_rl_repo")
import numpy as np

T, B, D = 2048, 2, 512
S = 32
NCORES = 8
TOK = T // NCORES            # 256 owned tokens per core
NT = 3                       # token tiles per core incl. halo block
NK = D // 128                # 4 K-tiles per projection

_CACHE = {}


def _build():
    import concourse.bacc as bacc
    import concourse.tile as tile
    import concourse.mybir as mybir

    F32 = mybir.dt.float32
    F32R = mybir.dt.float32r
    EXP = mybir.ActivationFunctionType.Exp
    SIG = mybir.ActivationFunctionType.Sigmoid

    nc = bacc.Bacc("TRN2", target_bir_lowering=False, debug=False,
                   num_devices=NCORES)

    key_ext = nc.dram_tensor("key_s", [NT * 128, B, D], F32, kind="ExternalInput")
    val_ext = nc.dram_tensor("value_s", [NT * 128, B, D], F32, kind="ExternalInput")
    qry_ext = nc.dram_tensor("query_s", [TOK, B, D], F32, kind="ExternalInput")
    wT_ext = nc.dram_tensor("wT", [4, D, D], F32, kind="ExternalInput")  # q,k,v,o
    b_ext = nc.dram_tensor("biases", [1, 4, D], F32, kind="ExternalInput")  # q,k,v,o
    c_ext = nc.dram_tensor("consts", [128, 272], F32, kind="ExternalInput")
    slab_ext = nc.dram_tensor("slab", [2, 128, 256], F32, kind="ExternalInput")
    neg_ext = nc.dram_tensor("negmask", [2, 256], F32, kind="ExternalInput")
    cm_ext = nc.dram_tensor("carrymask", [2, 2, 32, 128], F32, kind="ExternalInput")
    out_ext = nc.dram_tensor("out", [TOK, B, D], F32, kind="ExternalOutput")

    cs_dram = nc.dram_tensor("cs_local", [4, B * D], F32)
    gath_dram = nc.dram_tensor("cs_gath", [4 * NCORES, B * D], F32,
                               addr_space="Shared")
    barrier_in = nc.dram_tensor("barrier_in", [1, 4], F32)
    barrier_out = nc.dram_tensor("barrier_out", [NCORES, 4], F32,
                                 addr_space="Shared")

    with tile.TileContext(nc, num_cores=NCORES) as tc:
        with tc.tile_pool(name="consts", bufs=1) as cp, \
             tc.tile_pool(name="big", bufs=1) as bp, \
             tc.tile_pool(name="sc", bufs=3) as scp, \
             tc.tile_pool(name="psA", bufs=3, space="PSUM") as psA, \
             tc.tile_pool(name="psB", bufs=4, space="PSUM") as psB:

            # ---------------- constants / weights ----------------
            cext = cp.tile([128, 272], F32R)
            nc.sync.dma_start(out=cext, in_=c_ext[:, :].bitcast(F32R))
            ident = cext[:, 0:128]
            ones1 = cext[0:1, 144:272]

            # dummy collective at t=0: absorbs cross-core launch skew +
            # collectives-firmware warmup off the critical path (runs on the
            # TOPSP blocks while DMA/compute proceed).
            nc.gpsimd.collective_compute(
                "AllGather", mybir.AluOpType.bypass,
                replica_groups=[list(range(NCORES))],
                ins=[barrier_in.ap().opt()], outs=[barrier_out.ap().opt()])

            bias_sb = cp.tile([1, 4, 512], F32R)
            nc.sync.dma_start(out=bias_sb, in_=b_ext[:, :, :].bitcast(F32R))

            # owned-block key/value first (feeds the pre-collective path),
            # then the k/v weights, then everything else. Alternate the two
            # HWDGE issue engines (sync / scalar) to halve issue latency.
            key_sb = bp.tile([128, B, NT, 512], F32R)
            val_sb = bp.tile([128, B, NT, 512], F32R)
            qry_sb = bp.tile([128, B, 2, 512], F32R)
            wT_sb = cp.tile([128, 4, NK, 512], F32R)
            wT_src = wT_ext.ap().rearrange(
                "w (kt p) c -> p w kt c", p=128).bitcast(F32R)
            key_src = key_ext.ap().rearrange(
                "(tt p) b c -> p b tt c", p=128).bitcast(F32R)
            val_src = val_ext.ap().rearrange(
                "(tt p) b c -> p b tt c", p=128).bitcast(F32R)
            qry_src = qry_ext.ap().rearrange(
                "(tt p) b c -> p b tt c", p=128).bitcast(F32R)
            nc.sync.dma_start(out=key_sb[:, 0, 1:3, :], in_=key_src[:, 0, 1:3, :])
            nc.scalar.dma_start(out=val_sb[:, 0, 1:3, :], in_=val_src[:, 0, 1:3, :])
            nc.sync.dma_start(out=key_sb[:, 1, 1:3, :], in_=key_src[:, 1, 1:3, :])
            nc.scalar.dma_start(out=val_sb[:, 1, 1:3, :], in_=val_src[:, 1, 1:3, :])
            nc.sync.dma_start(out=wT_sb[:, 1, :, :], in_=wT_src[:, 1, :, :])
            nc.scalar.dma_start(out=wT_sb[:, 2, :, :], in_=wT_src[:, 2, :, :])
            nc.sync.dma_start(out=key_sb[:, 0, 0:1, :], in_=key_src[:, 0, 0:1, :])
            nc.scalar.dma_start(out=val_sb[:, 0, 0:1, :], in_=val_src[:, 0, 0:1, :])
            nc.sync.dma_start(out=key_sb[:, 1, 0:1, :], in_=key_src[:, 1, 0:1, :])
            nc.scalar.dma_start(out=val_sb[:, 1, 0:1, :], in_=val_src[:, 1, 0:1, :])

            slab_sb = bp.tile([128, 2, 256], F32)
            nc.sync.dma_start(out=slab_sb, in_=slab_ext.ap().rearrange(
                "j p c -> p j c"))
            negm_bc = bp.tile([128, 2, 256], F32)
            nc.scalar.dma_start(
                out=negm_bc,
                in_=neg_ext.ap().rearrange("(o j) c -> o j c", o=1).to_broadcast(
                    (128, 2, 256)))
            cmask_sb = cp.tile([32, 2, 2, 128], F32R)
            nc.sync.dma_start(out=cmask_sb, in_=cm_ext.ap().rearrange(
                "j k p c -> p j k c").bitcast(F32R))
            nc.scalar.dma_start(out=qry_sb[:, 0, :, :], in_=qry_src[:, 0, :, :])
            nc.sync.dma_start(out=qry_sb[:, 1, :, :], in_=qry_src[:, 1, :, :])
            nc.scalar.dma_start(out=wT_sb[:, 0, :, :], in_=wT_src[:, 0, :, :])
            nc.sync.dma_start(out=wT_sb[:, 3, :, :], in_=wT_src[:, 3, :, :])

            # transposed activations
            keyT = bp.tile([128, B, NT, NK, 128], F32R)
            valT = bp.tile([128, B, NT, NK, 128], F32R)
            qryT = bp.tile([128, B, 2, NK, 128], F32R)
            ek_sb = bp.tile([128, NT, B, 512], F32R)
            ekv_sb = bp.tile([128, NT, B, 512], F32R)
            sig_sb = bp.tile([128, 2, B, 512], F32)
            y_sb = bp.tile([128, 2, B, 512], F32R)
            yT = bp.tile([128, 2, B, NK, 128], F32R)
            W_sb = bp.tile([128, 2, 256], F32)
            WT_sb = bp.tile([128, 2, 2, 128], F32R)
            gath_sb = bp.tile([32, B * D], F32R)
            cs_sb = bp.tile([4, B * D], F32)

            # warm the ACT exp table (~2.7us load) before it's needed
            warm = scp.tile([1, 4], F32, tag="warm")
            nc.vector.memset(warm, 0.0)
            nc.scalar.activation(warm, warm, EXP)

            def transpose_in(src_sb, dst, b, tt):
                tp = psA.tile([128, 512], F32R, tag="t")
                for kt in range(NK):
                    nc.tensor.transpose(tp[:, kt * 128:(kt + 1) * 128],
                                        src_sb[:, b, tt, kt * 128:(kt + 1) * 128],
                                        ident)
                nc.vector.tensor_copy(dst[:, b, tt, :, :], tp)

            def project(actT, w, b, tt):
                """k/v/q projection for one (token-tile, batch): PSUM [128,512]."""
                pr = psA.tile([128, 512], F32, tag="t")
                for kt in range(NK):
                    nc.tensor.matmul(pr, actT[:, b, tt, kt, :],
                                     wT_sb[:, w, kt, :],
                                     start=(kt == 0), stop=False)
                nc.tensor.matmul(pr, ones1, bias_sb[0:1, w, :],
                                 start=False, stop=True)
                return pr

            # ------------- owned blocks first (tt=1,2): k,v -> ek,ekv -------------
            for tt in (1, 2, 0):           # owned, owned, halo
                for b in range(B):
                    transpose_in(key_sb, keyT, b, tt)
                    transpose_in(val_sb, valT, b, tt)
                    kp = project(keyT, 1, b, tt)
                    nc.scalar.activation(ek_sb[:, tt, b, :], kp, EXP)
                    vp = project(valT, 2, b, tt)
                    nc.vector.tensor_mul(ekv_sb[:, tt, b, :],
                                         ek_sb[:, tt, b, :].bitcast(F32), vp)
                if tt == 2:
                    # ---------------- colsums + collective ----------------
                    for c in range(B):
                        csp = psA.tile([4, 512], F32, tag="t")
                        for j in range(2):
                            for kind in range(2):
                                r = 2 * j + kind
                                sel = cext[:, 128 + 4 * r:128 + 4 * r + 4]
                                src = ekv_sb if kind == 0 else ek_sb
                                nc.tensor.matmul(csp, sel, src[:, j + 1, c, :],
                                                 start=(r == 0), stop=(r == 3))
                        nc.vector.tensor_copy(cs_sb[:, c * 512:(c + 1) * 512], csp)
                    nc.sync.dma_start(out=cs_dram[:, :], in_=cs_sb)
                    nc.gpsimd.collective_compute(
                        "AllGather", mybir.AluOpType.bypass,
                        replica_groups=[list(range(NCORES))],
                        ins=[cs_dram.ap().opt()], outs=[gath_dram.ap().opt()])
                    nc.sync.dma_start(out=gath_sb,
                                      in_=gath_dram[:, :].bitcast(F32R))

            # ---------------- W build (before sigmoid: keep ACT on Exp) ----------
            nc.gpsimd.affine_select(   # future -> -1e30   (keep tr+128-tc2 >= 0)
                out=slab_sb, in_=slab_sb, compare_op=mybir.AluOpType.is_ge,
                fill=-1e30, base=128, pattern=[[0, 2], [-1, 256]],
                channel_multiplier=1)
            nc.gpsimd.affine_select(   # ones-region -> 0  (keep tc2-97-tr >= 0)
                out=slab_sb, in_=slab_sb, compare_op=mybir.AluOpType.is_ge,
                fill=0.0, base=-97, pattern=[[0, 2], [1, 256]],
                channel_multiplier=-1)
            nc.vector.tensor_add(W_sb, slab_sb, negm_bc)
            WX = bp.tile([128, 2, 256], F32R)
            nc.scalar.activation(WX, W_sb, EXP)
            for j in range(2):
                wtp = psA.tile([128, 256], F32R, tag="t")
                for h in range(2):
                    nc.tensor.transpose(wtp[:, h * 128:(h + 1) * 128],
                                        WX[:, j, h * 128:(h + 1) * 128], ident)
                nc.vector.tensor_copy(WT_sb[:, j, :, :], wtp)

            # ---------------- q projection + sigmoid ----------------
            for b in range(B):
                for tt in range(2):
                    transpose_in(qry_sb, qryT, b, tt)
                    qp = project(qryT, 0, b, tt)
                    nc.scalar.activation(sig_sb[:, tt, b, :], qp, SIG)

            # ---------------- band matmuls + carry + y ----------------
            groups = [(0, 0), (0, 1), (1, 0), (1, 1)]   # (j, chunk=batch)

            def band_open(j, c):
                pn = psB.tile([128, 512], F32, tag="band")
                pd = psB.tile([128, 512], F32, tag="band")
                nc.tensor.matmul(pn, WT_sb[:, j, 0, :], ekv_sb[:, j, c, :],
                                 start=True, stop=False)
                nc.tensor.matmul(pn, WT_sb[:, j, 1, :], ekv_sb[:, j + 1, c, :],
                                 start=False, stop=False)
                nc.tensor.matmul(pd, WT_sb[:, j, 0, :], ek_sb[:, j, c, :],
                                 start=True, stop=False)
                nc.tensor.matmul(pd, WT_sb[:, j, 1, :], ek_sb[:, j + 1, c, :],
                                 start=False, stop=False)
                return pn, pd

            def band_carry(j, c, pn, pd):
                nc.tensor.matmul(pn, cmask_sb[:, j, 0, :],
                                 gath_sb[:, c * 512:(c + 1) * 512],
                                 start=False, stop=True)
                nc.tensor.matmul(pd, cmask_sb[:, j, 1, :],
                                 gath_sb[:, c * 512:(c + 1) * 512],
                                 start=False, stop=True)

            def band_y(j, c, pn, pd):
                rec = scp.tile([128, 512], F32, tag="rec")
                nc.vector.reciprocal_approx_fast(rec, pd)
                t1 = scp.tile([128, 512], F32, tag="t1")
                nc.vector.tensor_mul(t1, pn, rec)
                nc.vector.tensor_mul(y_sb[:, j, c, :], t1, sig_sb[:, j, c, :])

            def finish(j, c):
                """yT transpose + output projection + store for group (j,c)."""
                tp = psA.tile([128, 512], F32R, tag="t")
                for kt in range(NK):
                    nc.tensor.transpose(
                        tp[:, kt * 128:(kt + 1) * 128],
                        y_sb[:, j, c, kt * 128:(kt + 1) * 128], ident)
                nc.vector.tensor_copy(yT[:, j, c, :, :], tp)
                po = psA.tile([128, 512], F32, tag="t")
                for kt in range(NK):
                    nc.tensor.matmul(po, yT[:, j, c, kt, :],
                                     wT_sb[:, 3, kt, :],
                                     start=(kt == 0), stop=False)
                nc.tensor.matmul(po, ones1, bias_sb[0:1, 3, :],
                                 start=False, stop=True)
                ob = scp.tile([128, 512], F32, tag="ob")
                nc.vector.tensor_copy(ob, po)
                nc.sync.dma_start(
                    out=out_ext[j * 128:(j + 1) * 128, c, :], in_=ob)

            live = {}
            live[groups[0]] = band_open(*groups[0])
            live[groups[1]] = band_open(*groups[1])
            band_carry(*groups[0], *live[groups[0]])
            band_y(*groups[0], *live[groups[0]])
            live[groups[2]] = band_open(*groups[2])
            band_carry(*groups[1], *live[groups[1]])
            band_y(*groups[1], *live[groups[1]])
            finish(*groups[0])
            live[groups[3]] = band_open(*groups[3])
            band_carry(*groups[2], *live[groups[2]])
            band_y(*groups[2], *live[groups[2]])
            finish(*groups[1])
            band_carry(*groups[3], *live[groups[3]])
            band_y(*groups[3], *live[groups[3]])
            finish(*groups[2])
            finish(*groups[3])
    nc.compile()
    return nc


def _host_inputs(query, key, value, Wq, bq, Wk, bk, Wv, bv, pos_bias, Wo, bo):
    """Build the 8 per-core input maps (pure slicing/layout, no math)."""
    wT = np.ascontiguousarray(
        np.stack([Wq.T, Wk.T, Wv.T, Wo.T]).astype(np.float32))
    biases = np.ascontiguousarray(
        np.stack([bq, bk, bv, bo]).astype(np.float32)).reshape(1, 4, D)

    consts = np.zeros((128, 272), np.float32)
    consts[:, :128] = np.eye(128, dtype=np.float32)
    for r in range(4):
        consts[:, 128 + 4 * r + r] = 1.0
    consts[0, 144:272] = 1.0

    in_maps = []
    for i in range(NCORES):
        lo = TOK * i - 128
        key_s = np.zeros((NT * 128, B, D), np.float32)
        val_s = np.zeros((NT * 128, B, D), np.float32)
        src_lo = max(lo, 0)
        off = src_lo - lo
        key_s[off:] = key[src_lo:lo + NT * 128]
        val_s[off:] = value[src_lo:lo + NT * 128]
        qry_s = np.ascontiguousarray(query[TOK * i:TOK * (i + 1)]).astype(
            np.float32)

        slab = np.zeros((2, 128, 256), np.float32)
        negmask = np.zeros((2, 256), np.float32)
        for j in range(2):
            m = 2 * i + j
            c0 = 128 * (m - 1)
            clo = max(c0, 0)
            slab[j, :, clo - c0:] = pos_bias[128 * m:128 * (m + 1),
                                             clo:c0 + 256]
            if m == 0:
                negmask[j, :128] = -1e30
        carrymask = np.zeros((2, 2, 32, 128), np.float32)
        for j in range(2):
            m = 2 * i + j
            for kind in range(2):
                for r_ in range(32):
                    beta = 2 * (r_ // 4) + (r_ % 4) // 2
                    if r_ % 2 == kind and beta <= m - 2:
                        carrymask[j, kind, r_, :] = 1.0
        in_maps.append({
            "key_s": key_s, "value_s": val_s, "query_s": qry_s,
            "wT": wT, "biases": biases, "consts": consts,
            "slab": np.ascontiguousarray(slab), "negmask": negmask,
            "carrymask": carrymask,
        })
    return in_maps


def kernel(**inputs):
    from concourse.bass_utils import run_bass_kernel_spmd
    if "nc" not in _CACHE:
        _CACHE["nc"] = _build()
    nc = _CACHE["nc"]
    inputs = {k: np.asarray(v, dtype=np.float32) for k, v in inputs.items()}
    in_maps = _host_inputs(**inputs)
    res = run_bass_kernel_spmd(nc, in_maps, core_ids=list(range(NCORES)),
                               trace=False)
    out = np.concatenate([res.results[i]["out"] for i in range(NCORES)],
                         axis=0)
    return out.astype(np.float32)


# revision 8
# speedup vs baseline: 1.0937x; 1.0937x over previous
"""AFT-Local autoregressive attention on 8 Trainium2 NeuronCores.

Sequence-parallel decomposition: core i owns tokens [256i, 256i+256) (two
globally-aligned 128-token blocks m=2i, 2i+1) and recomputes the previous
128-token block as halo. With ek = exp(k) (the reference's max_logit cancels
in num/den), the AFT mixing for output block m is a banded matmul

    num[block m] = C_m (colsums of blocks <= m-2)  +  Wpair[m] @ [ekv[m-1]; ekv[m]]

where Wpair[tr, tc2] = 1 for tc2 <= tr+96, exp(pb) on the 32-band, 0 on the
future. C_m comes from one 8-core AllGather of per-block column sums
([4,1024] fp32 per core), folded in as a K=32 matmul. All matmuls run in
float32r (1 cycle/row).
"""
import sys
sys.path.insert(0, "/opt/trn_rl_repo")
import numpy as np

T, B, D = 2048, 2, 512
S = 32
NCORES = 8
TOK = T // NCORES            # 256 owned tokens per core
NT = 3                       # token tiles per core incl. halo block
NK = D // 128                # 4 K-tiles per projection

_CACHE = {}


def _build():
    import concourse.bacc as bacc
    import concourse.tile as tile
    import concourse.mybir as mybir

    F32 = mybir.dt.float32
    F32R = mybir.dt.float32r
    EXP = mybir.ActivationFunctionType.Exp
    SIG = mybir.ActivationFunctionType.Sigmoid

    nc = bacc.Bacc("TRN2", target_bir_lowering=False, debug=False,
                   num_devices=NCORES)

    key_ext = nc.dram_tensor("key_s", [NT * 128, B, D], F32, kind="ExternalInput")
    val_ext = nc.dram_tensor("value_s", [NT * 128, B, D], F32, kind="ExternalInput")
    qry_ext = nc.dram_tensor("query_s", [TOK, B, D], F32, kind="ExternalInput")
    wT_ext = nc.dram_tensor("wT", [4, D, D], F32, kind="ExternalInput")  # q,k,v,o
    b_ext = nc.dram_tensor("biases", [1, 4, D], F32, kind="ExternalInput")  # q,k,v,o
    c_ext = nc.dram_tensor("consts", [128, 272], F32, kind="ExternalInput")
    slab_ext = nc.dram_tensor("slab", [2, 128, 256], F32, kind="ExternalInput")
    neg_ext = nc.dram_tensor("negmask", [2, 256], F32, kind="ExternalInput")
    cm_ext = nc.dram_tensor("carrymask", [2, 2, 32, 128], F32, kind="ExternalInput")
    out_ext = nc.dram_tensor("out", [TOK, B, D], F32, kind="ExternalOutput")

    cs_dram = nc.dram_tensor("cs_local", [4, B * D], F32)
    gath_dram = nc.dram_tensor("cs_gath", [4 * NCORES, B * D], F32,
                               addr_space="Shared")
    barrier_in = nc.dram_tensor("barrier_in", [1, 4], F32)
    barrier_out = nc.dram_tensor("barrier_out", [NCORES, 4], F32,
                                 addr_space="Shared")

    with tile.TileContext(nc, num_cores=NCORES) as tc:
        with tc.tile_pool(name="consts", bufs=1) as cp, \
             tc.tile_pool(name="big", bufs=1) as bp, \
             tc.tile_pool(name="sc", bufs=3) as scp, \
             tc.tile_pool(name="psA", bufs=3, space="PSUM") as psA, \
             tc.tile_pool(name="psB", bufs=4, space="PSUM") as psB:

            # ---------------- constants / weights ----------------
            cext = cp.tile([128, 272], F32R)
            nc.sync.dma_start(out=cext, in_=c_ext[:, :].bitcast(F32R))
            ident = cext[:, 0:128]
            ones1 = cext[0:1, 144:272]

            # dummy collective at t=0: absorbs cross-core launch skew +
            # collectives-firmware warmup off the critical path (runs on the
            # TOPSP blocks while DMA/compute proceed).
            nc.gpsimd.collective_compute(
                "AllGather", mybir.AluOpType.bypass,
                replica_groups=[list(range(NCORES))],
                ins=[barrier_in.ap().opt()], outs=[barrier_out.ap().opt()])

            bias_sb = cp.tile([1, 4, 512], F32R)
            nc.sync.dma_start(out=bias_sb, in_=b_ext[:, :, :].bitcast(F32R))
            # exp(bk) cancels in num/den; bv resurfaces as y += sig*bv after
            # the division; bo is added on the output copy. Only bq needs a
            # K=1 bias matmul.
            bvo_bc = cp.tile([128, 2, 512], F32)
            nc.scalar.dma_start(out=bvo_bc,
                                in_=b_ext[:, 2:4, :].to_broadcast((128, 2, 512)))

            # owned-block key/value first (feeds the pre-collective path),
            # then the k/v weights, then everything else. Alternate the two
            # HWDGE issue engines (sync / scalar) to halve issue latency.
            key_sb = bp.tile([128, B, NT, 512], F32R)
            val_sb = bp.tile([128, B, NT, 512], F32R)
            qry_sb = bp.tile([128, B, 2, 512], F32R)
            wT_sb = cp.tile([128, 4, NK, 512], F32R)
            wT_src = wT_ext.ap().rearrange(
                "w (kt p) c -> p w kt c", p=128).bitcast(F32R)
            key_src = key_ext.ap().rearrange(
                "(tt p) b c -> p b tt c", p=128).bitcast(F32R)
            val_src = val_ext.ap().rearrange(
                "(tt p) b c -> p b tt c", p=128).bitcast(F32R)
            qry_src = qry_ext.ap().rearrange(
                "(tt p) b c -> p b tt c", p=128).bitcast(F32R)
            nc.sync.dma_start(out=key_sb[:, 0, 1:3, :], in_=key_src[:, 0, 1:3, :])
            nc.scalar.dma_start(out=val_sb[:, 0, 1:3, :], in_=val_src[:, 0, 1:3, :])
            nc.sync.dma_start(out=key_sb[:, 1, 1:3, :], in_=key_src[:, 1, 1:3, :])
            nc.scalar.dma_start(out=val_sb[:, 1, 1:3, :], in_=val_src[:, 1, 1:3, :])
            nc.sync.dma_start(out=wT_sb[:, 1, :, :], in_=wT_src[:, 1, :, :])
            nc.scalar.dma_start(out=wT_sb[:, 2, :, :], in_=wT_src[:, 2, :, :])
            nc.sync.dma_start(out=key_sb[:, 0, 0:1, :], in_=key_src[:, 0, 0:1, :])
            nc.scalar.dma_start(out=val_sb[:, 0, 0:1, :], in_=val_src[:, 0, 0:1, :])
            nc.sync.dma_start(out=key_sb[:, 1, 0:1, :], in_=key_src[:, 1, 0:1, :])
            nc.scalar.dma_start(out=val_sb[:, 1, 0:1, :], in_=val_src[:, 1, 0:1, :])

            slab_sb = bp.tile([128, 2, 256], F32)
            nc.sync.dma_start(out=slab_sb, in_=slab_ext.ap().rearrange(
                "j p c -> p j c"))
            negm_bc = bp.tile([128, 2, 256], F32)
            nc.scalar.dma_start(
                out=negm_bc,
                in_=neg_ext.ap().rearrange("(o j) c -> o j c", o=1).to_broadcast(
                    (128, 2, 256)))
            cmask_sb = cp.tile([32, 2, 2, 128], F32R)
            nc.sync.dma_start(out=cmask_sb, in_=cm_ext.ap().rearrange(
                "j k p c -> p j k c").bitcast(F32R))
            nc.scalar.dma_start(out=qry_sb[:, 0, :, :], in_=qry_src[:, 0, :, :])
            nc.sync.dma_start(out=qry_sb[:, 1, :, :], in_=qry_src[:, 1, :, :])
            nc.scalar.dma_start(out=wT_sb[:, 0, :, :], in_=wT_src[:, 0, :, :])
            nc.sync.dma_start(out=wT_sb[:, 3, :, :], in_=wT_src[:, 3, :, :])

            # transposed activations
            keyT = bp.tile([128, B, NT, NK, 128], F32R)
            valT = bp.tile([128, B, NT, NK, 128], F32R)
            qryT = bp.tile([128, B, 2, NK, 128], F32R)
            ek_sb = bp.tile([128, NT, B, 512], F32R)
            ekv_sb = bp.tile([128, NT, B, 512], F32R)
            sig_sb = bp.tile([128, 2, B, 512], F32)
            y_sb = bp.tile([128, 2, B, 512], F32R)
            yT = bp.tile([128, 2, B, NK, 128], F32R)
            W_sb = bp.tile([128, 2, 256], F32)
            WT_sb = bp.tile([128, 2, 2, 128], F32R)
            gath_sb = bp.tile([32, B * D], F32R)
            cs_sb = bp.tile([4, B * D], F32)

            # warm the ACT exp table (~2.7us load) before it's needed
            warm = scp.tile([1, 4], F32, tag="warm")
            nc.vector.memset(warm, 0.0)
            nc.scalar.activation(warm, warm, EXP)

            tin_ctr = [0]

            def transpose_in(src_sb, dst, b, tt):
                tp = psA.tile([128, 512], F32R, tag="t")
                for kt in range(NK):
                    nc.tensor.transpose(tp[:, kt * 128:(kt + 1) * 128],
                                        src_sb[:, b, tt, kt * 128:(kt + 1) * 128],
                                        ident)
                tin_ctr[0] += 1
                if tin_ctr[0] % 2:
                    nc.vector.tensor_copy(dst[:, b, tt, :, :], tp)
                else:
                    nc.scalar.copy(dst[:, b, tt, :, :], tp)

            def project(actT, w, b, tt):
                """k/v/q projection for one (token-tile, batch): PSUM [128,512].
                Bias matmul only for q: bk cancels, bv/bo folded in later."""
                pr = psA.tile([128, 512], F32, tag="t")
                for kt in range(NK):
                    nc.tensor.matmul(pr, actT[:, b, tt, kt, :],
                                     wT_sb[:, w, kt, :],
                                     start=(kt == 0), stop=(w != 0 and kt == NK - 1))
                if w == 0:
                    nc.tensor.matmul(pr, ones1, bias_sb[0:1, 0, :],
                                     start=False, stop=True)
                return pr

            # ------------- owned blocks first (tt=1,2): k,v -> ek,ekv -------------
            for tt in (1, 2, 0):           # owned, owned, halo
                for b in range(B):
                    transpose_in(key_sb, keyT, b, tt)
                    transpose_in(val_sb, valT, b, tt)
                    kp = project(keyT, 1, b, tt)
                    nc.scalar.activation(ek_sb[:, tt, b, :], kp, EXP)
                    vp = project(valT, 2, b, tt)
                    nc.vector.tensor_mul(ekv_sb[:, tt, b, :],
                                         ek_sb[:, tt, b, :].bitcast(F32), vp)
                if tt == 2:
                    # ---------------- colsums + collective ----------------
                    for c in range(B):
                        csp = psA.tile([4, 512], F32, tag="t")
                        for j in range(2):
                            for kind in range(2):
                                r = 2 * j + kind
                                sel = cext[:, 128 + 4 * r:128 + 4 * r + 4]
                                src = ekv_sb if kind == 0 else ek_sb
                                nc.tensor.matmul(csp, sel, src[:, j + 1, c, :],
                                                 start=(r == 0), stop=(r == 3))
                        nc.vector.tensor_copy(cs_sb[:, c * 512:(c + 1) * 512], csp)
                    nc.sync.dma_start(out=cs_dram[:, :], in_=cs_sb)
                    nc.gpsimd.collective_compute(
                        "AllGather", mybir.AluOpType.bypass,
                        replica_groups=[list(range(NCORES))],
                        ins=[cs_dram.ap().opt()], outs=[gath_dram.ap().opt()])
                    nc.sync.dma_start(out=gath_sb,
                                      in_=gath_dram[:, :].bitcast(F32R))

            # ---------------- W build (before sigmoid: keep ACT on Exp) ----------
            nc.gpsimd.affine_select(   # future -> -1e30   (keep tr+128-tc2 >= 0)
                out=slab_sb, in_=slab_sb, compare_op=mybir.AluOpType.is_ge,
                fill=-1e30, base=128, pattern=[[0, 2], [-1, 256]],
                channel_multiplier=1)
            nc.gpsimd.affine_select(   # ones-region -> 0  (keep tc2-97-tr >= 0)
                out=slab_sb, in_=slab_sb, compare_op=mybir.AluOpType.is_ge,
                fill=0.0, base=-97, pattern=[[0, 2], [1, 256]],
                channel_multiplier=-1)
            nc.vector.tensor_add(W_sb, slab_sb, negm_bc)
            WX = bp.tile([128, 2, 256], F32R)
            nc.scalar.activation(WX, W_sb, EXP)
            for j in range(2):
                wtp = psA.tile([128, 256], F32R, tag="t")
                for h in range(2):
                    nc.tensor.transpose(wtp[:, h * 128:(h + 1) * 128],
                                        WX[:, j, h * 128:(h + 1) * 128], ident)
                nc.vector.tensor_copy(WT_sb[:, j, :, :], wtp)

            # ---------------- q projection + sigmoid ----------------
            for b in range(B):
                for tt in range(2):
                    transpose_in(qry_sb, qryT, b, tt)
                    qp = project(qryT, 0, b, tt)
                    nc.scalar.activation(sig_sb[:, tt, b, :], qp, SIG)

            # ---------------- band matmuls + carry + y ----------------
            groups = [(0, 0), (0, 1), (1, 0), (1, 1)]   # (j, chunk=batch)

            def band_open(j, c):
                pn = psB.tile([128, 512], F32, tag="band")
                pd = psB.tile([128, 512], F32, tag="band")
                nc.tensor.matmul(pn, WT_sb[:, j, 0, :], ekv_sb[:, j, c, :],
                                 start=True, stop=False)
                nc.tensor.matmul(pn, WT_sb[:, j, 1, :], ekv_sb[:, j + 1, c, :],
                                 start=False, stop=False)
                nc.tensor.matmul(pd, WT_sb[:, j, 0, :], ek_sb[:, j, c, :],
                                 start=True, stop=False)
                nc.tensor.matmul(pd, WT_sb[:, j, 1, :], ek_sb[:, j + 1, c, :],
                                 start=False, stop=False)
                return pn, pd

            def band_carry(j, c, pn, pd):
                nc.tensor.matmul(pn, cmask_sb[:, j, 0, :],
                                 gath_sb[:, c * 512:(c + 1) * 512],
                                 start=False, stop=True)
                nc.tensor.matmul(pd, cmask_sb[:, j, 1, :],
                                 gath_sb[:, c * 512:(c + 1) * 512],
                                 start=False, stop=True)

            def band_y(j, c, pn, pd):
                rec = scp.tile([128, 512], F32, tag="rec")
                nc.vector.reciprocal_approx_fast(rec, pd)
                t1 = scp.tile([128, 512], F32, tag="t1")
                nc.vector.tensor_mul(t1, pn, rec)
                nc.vector.tensor_add(t1, t1, bvo_bc[:, 0, :])
                nc.vector.tensor_mul(y_sb[:, j, c, :], t1, sig_sb[:, j, c, :])

            def finish(j, c):
                """yT transpose + output projection + store for group (j,c)."""
                tp = psA.tile([128, 512], F32R, tag="t")
                for kt in range(NK):
                    nc.tensor.transpose(
                        tp[:, kt * 128:(kt + 1) * 128],
                        y_sb[:, j, c, kt * 128:(kt + 1) * 128], ident)
                nc.vector.tensor_copy(yT[:, j, c, :, :], tp)
                po = psA.tile([128, 512], F32, tag="t")
                for kt in range(NK):
                    nc.tensor.matmul(po, yT[:, j, c, kt, :],
                                     wT_sb[:, 3, kt, :],
                                     start=(kt == 0), stop=(kt == NK - 1))
                ob = scp.tile([128, 512], F32, tag="ob")
                nc.vector.tensor_add(ob, po, bvo_bc[:, 1, :])
                nc.sync.dma_start(
                    out=out_ext[j * 128:(j + 1) * 128, c, :], in_=ob)

            live = {}
            live[groups[0]] = band_open(*groups[0])
            live[groups[1]] = band_open(*groups[1])
            band_carry(*groups[0], *live[groups[0]])
            band_y(*groups[0], *live[groups[0]])
            live[groups[2]] = band_open(*groups[2])
            band_carry(*groups[1], *live[groups[1]])
            band_y(*groups[1], *live[groups[1]])
            finish(*groups[0])
            live[groups[3]] = band_open(*groups[3])
            band_carry(*groups[2], *live[groups[2]])
            band_y(*groups[2], *live[groups[2]])
            finish(*groups[1])
            band_carry(*groups[3], *live[groups[3]])
            band_y(*groups[3], *live[groups[3]])
            finish(*groups[2])
            finish(*groups[3])
    nc.compile()
    return nc


def _host_inputs(query, key, value, Wq, bq, Wk, bk, Wv, bv, pos_bias, Wo, bo):
    """Build the 8 per-core input maps (pure slicing/layout, no math)."""
    wT = np.ascontiguousarray(
        np.stack([Wq.T, Wk.T, Wv.T, Wo.T]).astype(np.float32))
    biases = np.ascontiguousarray(
        np.stack([bq, bk, bv, bo]).astype(np.float32)).reshape(1, 4, D)

    consts = np.zeros((128, 272), np.float32)
    consts[:, :128] = np.eye(128, dtype=np.float32)
    for r in range(4):
        consts[:, 128 + 4 * r + r] = 1.0
    consts[0, 144:272] = 1.0

    in_maps = []
    for i in range(NCORES):
        lo = TOK * i - 128
        key_s = np.zeros((NT * 128, B, D), np.float32)
        val_s = np.zeros((NT * 128, B, D), np.float32)
        src_lo = max(lo, 0)
        off = src_lo - lo
        key_s[off:] = key[src_lo:lo + NT * 128]
        val_s[off:] = value[src_lo:lo + NT * 128]
        qry_s = np.ascontiguousarray(query[TOK * i:TOK * (i + 1)]).astype(
            np.float32)

        slab = np.zeros((2, 128, 256), np.float32)
        negmask = np.zeros((2, 256), np.float32)
        for j in range(2):
            m = 2 * i + j
            c0 = 128 * (m - 1)
            clo = max(c0, 0)
            slab[j, :, clo - c0:] = pos_bias[128 * m:128 * (m + 1),
                                             clo:c0 + 256]
            if m == 0:
                negmask[j, :128] = -1e30
        carrymask = np.zeros((2, 2, 32, 128), np.float32)
        for j in range(2):
            m = 2 * i + j
            for kind in range(2):
                for r_ in range(32):
                    beta = 2 * (r_ // 4) + (r_ % 4) // 2
                    if r_ % 2 == kind and beta <= m - 2:
                        carrymask[j, kind, r_, :] = 1.0
        in_maps.append({
            "key_s": key_s, "value_s": val_s, "query_s": qry_s,
            "wT": wT, "biases": biases, "consts": consts,
            "slab": np.ascontiguousarray(slab), "negmask": negmask,
            "carrymask": carrymask,
        })
    return in_maps


def kernel(**inputs):
    from concourse.bass_utils import run_bass_kernel_spmd
    if "nc" not in _CACHE:
        _CACHE["nc"] = _build()
    nc = _CACHE["nc"]
    inputs = {k: np.asarray(v, dtype=np.float32) for k, v in inputs.items()}
    in_maps = _host_inputs(**inputs)
    res = run_bass_kernel_spmd(nc, in_maps, core_ids=list(range(NCORES)),
                               trace=False)
    out = np.concatenate([res.results[i]["out"] for i in range(NCORES)],
                         axis=0)
    return out.astype(np.float32)


# revision 9
# speedup vs baseline: 1.2742x; 1.1650x over previous
"""AFT-Local autoregressive attention on 8 Trainium2 NeuronCores.

Sequence-parallel decomposition: core i owns tokens [256i, 256i+256) (two
globally-aligned 128-token blocks m=2i, 2i+1) and recomputes the previous
128-token block as halo. With ek = exp(k) (the reference's max_logit cancels
in num/den; so do bk, and bv resurfaces as y += sig*bv after the division),
the AFT mixing for output block m is a banded matmul

    num[block m] = C_m (colsums of blocks <= m-2)  +  Wpair[m] @ [ekv[m-1]; ekv[m]]

where Wpair[tr, tc2] = 1 for tc2 <= tr+96, exp(pb) on the 32-band, 0 on the
future. C_m comes from one 8-core AllGather of per-block column sums
([4,1024] fp32 per core), folded in as a K=32 matmul. Matmuls run in bf16
(fp32 PSUM accumulate); a dummy AllGather at t=0 absorbs core launch skew.
"""
import sys
sys.path.insert(0, "/opt/trn_rl_repo")
import numpy as np

T, B, D = 2048, 2, 512
S = 32
NCORES = 8
TOK = T // NCORES            # 256 owned tokens per core
NT = 3                       # token tiles per core incl. halo block
NK = D // 128                # 4 K-tiles per projection

_CACHE = {}


def _build():
    import concourse.bacc as bacc
    import concourse.tile as tile
    import concourse.mybir as mybir

    F32 = mybir.dt.float32
    BF16 = mybir.dt.bfloat16
    EXP = mybir.ActivationFunctionType.Exp
    SIG = mybir.ActivationFunctionType.Sigmoid

    nc = bacc.Bacc("TRN2", target_bir_lowering=False, debug=False,
                   num_devices=NCORES)

    key_ext = nc.dram_tensor("key_s", [NT * 128, B, D], F32, kind="ExternalInput")
    val_ext = nc.dram_tensor("value_s", [NT * 128, B, D], F32, kind="ExternalInput")
    qry_ext = nc.dram_tensor("query_s", [TOK, B, D], F32, kind="ExternalInput")
    wT_ext = nc.dram_tensor("wT", [4, D, D], F32, kind="ExternalInput")  # q,k,v,o
    b_ext = nc.dram_tensor("biases", [1, 4, D], F32, kind="ExternalInput")
    c_ext = nc.dram_tensor("consts", [128, 272], F32, kind="ExternalInput")
    slab_ext = nc.dram_tensor("slab", [2, 128, 256], F32, kind="ExternalInput")
    neg_ext = nc.dram_tensor("negmask", [2, 256], F32, kind="ExternalInput")
    cm_ext = nc.dram_tensor("carrymask", [2, 2, 32, 128], F32, kind="ExternalInput")
    out_ext = nc.dram_tensor("out", [TOK, B, D], F32, kind="ExternalOutput")

    cs_dram = nc.dram_tensor("cs_local", [4, B * D], F32)
    gath_dram = nc.dram_tensor("cs_gath", [4 * NCORES, B * D], F32,
                               addr_space="Shared")
    barrier_in = nc.dram_tensor("barrier_in", [1, 4], F32)
    barrier_out = nc.dram_tensor("barrier_out", [NCORES, 4], F32,
                                 addr_space="Shared")

    with tile.TileContext(nc, num_cores=NCORES) as tc:
        with tc.tile_pool(name="consts", bufs=1) as cp, \
             tc.tile_pool(name="big", bufs=1) as bp, \
             tc.tile_pool(name="sc", bufs=3) as scp, \
             tc.tile_pool(name="psA", bufs=3, space="PSUM") as psA, \
             tc.tile_pool(name="psB", bufs=4, space="PSUM") as psB:

            # dummy collective at t=0: absorbs cross-core launch skew +
            # collectives-firmware warmup off the critical path.
            nc.gpsimd.collective_compute(
                "AllGather", mybir.AluOpType.bypass,
                replica_groups=[list(range(NCORES))],
                ins=[barrier_in.ap().opt()], outs=[barrier_out.ap().opt()])

            # ---------------- input DMAs (SWDGE casts f32 -> bf16) ----------
            key_sb = bp.tile([128, B, NT, 512], BF16)
            val_sb = bp.tile([128, B, NT, 512], BF16)
            qry_sb = bp.tile([128, B, 2, 512], BF16)
            wT_sb = cp.tile([128, 4, NK, 512], BF16)
            wT_src = wT_ext.ap().rearrange("w (kt p) c -> p w kt c", p=128)
            key_src = key_ext.ap().rearrange("(tt p) b c -> p b tt c", p=128)
            val_src = val_ext.ap().rearrange("(tt p) b c -> p b tt c", p=128)
            qry_src = qry_ext.ap().rearrange("(tt p) b c -> p b tt c", p=128)
            nc.gpsimd.dma_start(out=key_sb, in_=key_src)
            nc.gpsimd.dma_start(out=val_sb, in_=val_src)
            nc.gpsimd.dma_start(out=wT_sb[:, 1, :, :], in_=wT_src[:, 1, :, :])
            nc.gpsimd.dma_start(out=wT_sb[:, 2, :, :], in_=wT_src[:, 2, :, :])
            cext = cp.tile([128, 272], BF16)
            nc.gpsimd.dma_start(out=cext, in_=c_ext[:, :])
            ident = cext[:, 0:128]
            ones1 = cext[0:1, 144:272]
            bias_sb = cp.tile([1, 4, 512], BF16)
            nc.gpsimd.dma_start(out=bias_sb, in_=b_ext[:, :, :])
            nc.gpsimd.dma_start(out=qry_sb, in_=qry_src)
            nc.gpsimd.dma_start(out=wT_sb[:, 0, :, :], in_=wT_src[:, 0, :, :])
            nc.gpsimd.dma_start(out=wT_sb[:, 3, :, :], in_=wT_src[:, 3, :, :])
            cmask_sb = cp.tile([32, 2, 2, 128], BF16)
            nc.gpsimd.dma_start(out=cmask_sb, in_=cm_ext.ap().rearrange(
                "j k p c -> p j k c"))

            # f32 side data (HWDGE)
            bvo_bc = cp.tile([128, 2, 512], F32)
            nc.scalar.dma_start(out=bvo_bc,
                                in_=b_ext[:, 2:4, :].to_broadcast((128, 2, 512)))
            slab_sb = bp.tile([128, 2, 256], F32)
            nc.sync.dma_start(out=slab_sb, in_=slab_ext.ap().rearrange(
                "j p c -> p j c"))
            negm_bc = bp.tile([128, 2, 256], F32)
            nc.scalar.dma_start(
                out=negm_bc,
                in_=neg_ext.ap().rearrange("(o j) c -> o j c", o=1).to_broadcast(
                    (128, 2, 256)))

            # transposed activations / working tiles
            keyT = bp.tile([128, B, NT, NK, 128], BF16)
            valT = bp.tile([128, B, NT, NK, 128], BF16)
            qryT = bp.tile([128, B, 2, NK, 128], BF16)
            ek_sb = bp.tile([128, NT, B, 512], BF16)
            ekv_sb = bp.tile([128, NT, B, 512], BF16)
            sig_sb = bp.tile([128, 2, B, 512], F32)
            y_sb = bp.tile([128, 2, B, 512], BF16)
            yT = bp.tile([128, 2, B, NK, 128], BF16)
            W_sb = bp.tile([128, 2, 256], F32)
            WT_sb = bp.tile([128, 2, 2, 128], BF16)
            gath_sb = bp.tile([32, B * D], BF16)
            cs_sb = bp.tile([4, B * D], F32)

            # warm the ACT exp table (~2.7us load) before it's needed
            warm = scp.tile([1, 4], F32, tag="warm")
            nc.vector.memset(warm, 0.0)
            nc.scalar.activation(warm, warm, EXP)

            tin_ctr = [0]

            def transpose_in(src_sb, dst, b, tt):
                tp = psA.tile([128, 512], BF16, tag="t")
                for kt in range(NK):
                    nc.tensor.transpose(tp[:, kt * 128:(kt + 1) * 128],
                                        src_sb[:, b, tt, kt * 128:(kt + 1) * 128],
                                        ident)
                tin_ctr[0] += 1
                if tin_ctr[0] % 2:
                    nc.vector.tensor_copy(dst[:, b, tt, :, :], tp)
                else:
                    nc.scalar.copy(dst[:, b, tt, :, :], tp)

            def project(actT, w, b, tt):
                """k/v/q projection for one (token-tile, batch): PSUM [128,512].
                Bias matmul only for q: bk cancels, bv/bo folded in later."""
                pr = psA.tile([128, 512], F32, tag="t")
                for kt in range(NK):
                    nc.tensor.matmul(pr, actT[:, b, tt, kt, :],
                                     wT_sb[:, w, kt, :],
                                     start=(kt == 0), stop=(w != 0 and kt == NK - 1))
                if w == 0:
                    nc.tensor.matmul(pr, ones1, bias_sb[0:1, 0, :],
                                     start=False, stop=True)
                return pr

            # ------------- owned blocks first (tt=1,2): k,v -> ek,ekv --------
            for tt in (1, 2, 0):           # owned, owned, halo
                for b in range(B):
                    transpose_in(key_sb, keyT, b, tt)
                    transpose_in(val_sb, valT, b, tt)
                    kp = project(keyT, 1, b, tt)
                    nc.scalar.activation(ek_sb[:, tt, b, :], kp, EXP)
                    vp = project(valT, 2, b, tt)
                    nc.vector.tensor_mul(ekv_sb[:, tt, b, :],
                                         ek_sb[:, tt, b, :], vp)
                if tt == 2:
                    # ---------------- colsums + collective ----------------
                    for c in range(B):
                        csp = psA.tile([4, 512], F32, tag="t")
                        for j in range(2):
                            for kind in range(2):
                                r = 2 * j + kind
                                sel = cext[:, 128 + 4 * r:128 + 4 * r + 4]
                                src = ekv_sb if kind == 0 else ek_sb
                                nc.tensor.matmul(csp, sel, src[:, j + 1, c, :],
                                                 start=(r == 0), stop=(r == 3))
                        nc.vector.tensor_copy(cs_sb[:, c * 512:(c + 1) * 512], csp)
                    nc.sync.dma_start(out=cs_dram[:, :], in_=cs_sb)
                    nc.gpsimd.collective_compute(
                        "AllGather", mybir.AluOpType.bypass,
                        replica_groups=[list(range(NCORES))],
                        ins=[cs_dram.ap().opt()], outs=[gath_dram.ap().opt()])
                    nc.gpsimd.dma_start(out=gath_sb, in_=gath_dram[:, :])

            # ---------------- W build (before sigmoid: keep ACT on Exp) ------
            nc.gpsimd.affine_select(   # future -> -1e30   (keep tr+128-tc2 >= 0)
                out=slab_sb, in_=slab_sb, compare_op=mybir.AluOpType.is_ge,
                fill=-1e30, base=128, pattern=[[0, 2], [-1, 256]],
                channel_multiplier=1)
            nc.gpsimd.affine_select(   # ones-region -> 0  (keep tc2-97-tr >= 0)
                out=slab_sb, in_=slab_sb, compare_op=mybir.AluOpType.is_ge,
                fill=0.0, base=-97, pattern=[[0, 2], [1, 256]],
                channel_multiplier=-1)
            nc.vector.tensor_add(W_sb, slab_sb, negm_bc)
            WX = bp.tile([128, 2, 256], BF16)
            nc.scalar.activation(WX, W_sb, EXP)
            for j in range(2):
                wtp = psA.tile([128, 256], BF16, tag="t")
                for h in range(2):
                    nc.tensor.transpose(wtp[:, h * 128:(h + 1) * 128],
                                        WX[:, j, h * 128:(h + 1) * 128], ident)
                nc.vector.tensor_copy(WT_sb[:, j, :, :], wtp)

            # ---------------- q projection + sigmoid ----------------
            for b in range(B):
                for tt in range(2):
                    transpose_in(qry_sb, qryT, b, tt)
                    qp = project(qryT, 0, b, tt)
                    nc.scalar.activation(sig_sb[:, tt, b, :], qp, SIG)

            # ---------------- band matmuls + carry + y ----------------
            groups = [(0, 0), (0, 1), (1, 0), (1, 1)]   # (j, chunk=batch)

            def band_open(j, c):
                pn = psB.tile([128, 512], F32, tag="band")
                pd = psB.tile([128, 512], F32, tag="band")
                nc.tensor.matmul(pn, WT_sb[:, j, 0, :], ekv_sb[:, j, c, :],
                                 start=True, stop=False)
                nc.tensor.matmul(pn, WT_sb[:, j, 1, :], ekv_sb[:, j + 1, c, :],
                                 start=False, stop=False)
                nc.tensor.matmul(pd, WT_sb[:, j, 0, :], ek_sb[:, j, c, :],
                                 start=True, stop=False)
                nc.tensor.matmul(pd, WT_sb[:, j, 1, :], ek_sb[:, j + 1, c, :],
                                 start=False, stop=False)
                return pn, pd

            def band_carry(j, c, pn, pd):
                nc.tensor.matmul(pn, cmask_sb[:, j, 0, :],
                                 gath_sb[:, c * 512:(c + 1) * 512],
                                 start=False, stop=True)
                nc.tensor.matmul(pd, cmask_sb[:, j, 1, :],
                                 gath_sb[:, c * 512:(c + 1) * 512],
                                 start=False, stop=True)

            def band_y(j, c, pn, pd):
                rec = scp.tile([128, 512], F32, tag="rec")
                nc.vector.reciprocal_approx_fast(rec, pd)
                t1 = scp.tile([128, 512], F32, tag="t1")
                nc.vector.tensor_mul(t1, pn, rec)
                nc.vector.tensor_add(t1, t1, bvo_bc[:, 0, :])
                nc.vector.tensor_mul(y_sb[:, j, c, :], t1, sig_sb[:, j, c, :])

            def finish(j, c):
                """yT transpose + output projection + store for group (j,c)."""
                tp = psA.tile([128, 512], BF16, tag="t")
                for kt in range(NK):
                    nc.tensor.transpose(
                        tp[:, kt * 128:(kt + 1) * 128],
                        y_sb[:, j, c, kt * 128:(kt + 1) * 128], ident)
                nc.vector.tensor_copy(yT[:, j, c, :, :], tp)
                po = psA.tile([128, 512], F32, tag="t")
                for kt in range(NK):
                    nc.tensor.matmul(po, yT[:, j, c, kt, :],
                                     wT_sb[:, 3, kt, :],
                                     start=(kt == 0), stop=(kt == NK - 1))
                ob = scp.tile([128, 512], F32, tag="ob")
                nc.vector.tensor_add(ob, po, bvo_bc[:, 1, :])
                nc.sync.dma_start(
                    out=out_ext[j * 128:(j + 1) * 128, c, :], in_=ob)

            live = {}
            live[groups[0]] = band_open(*groups[0])
            live[groups[1]] = band_open(*groups[1])
            band_carry(*groups[0], *live[groups[0]])
            band_y(*groups[0], *live[groups[0]])
            live[groups[2]] = band_open(*groups[2])
            band_carry(*groups[1], *live[groups[1]])
            band_y(*groups[1], *live[groups[1]])
            finish(*groups[0])
            live[groups[3]] = band_open(*groups[3])
            band_carry(*groups[2], *live[groups[2]])
            band_y(*groups[2], *live[groups[2]])
            finish(*groups[1])
            band_carry(*groups[3], *live[groups[3]])
            band_y(*groups[3], *live[groups[3]])
            finish(*groups[2])
            finish(*groups[3])
    nc.compile()
    return nc


def _host_inputs(query, key, value, Wq, bq, Wk, bk, Wv, bv, pos_bias, Wo, bo):
    """Build the 8 per-core input maps (pure slicing/layout, no math)."""
    wT = np.ascontiguousarray(
        np.stack([Wq.T, Wk.T, Wv.T, Wo.T]).astype(np.float32))
    biases = np.ascontiguousarray(
        np.stack([bq, bk, bv, bo]).astype(np.float32)).reshape(1, 4, D)

    consts = np.zeros((128, 272), np.float32)
    consts[:, :128] = np.eye(128, dtype=np.float32)
    for r in range(4):
        consts[:, 128 + 4 * r + r] = 1.0
    consts[0, 144:272] = 1.0

    in_maps = []
    for i in range(NCORES):
        lo = TOK * i - 128
        key_s = np.zeros((NT * 128, B, D), np.float32)
        val_s = np.zeros((NT * 128, B, D), np.float32)
        src_lo = max(lo, 0)
        off = src_lo - lo
        key_s[off:] = key[src_lo:lo + NT * 128]
        val_s[off:] = value[src_lo:lo + NT * 128]
        qry_s = np.ascontiguousarray(query[TOK * i:TOK * (i + 1)]).astype(
            np.float32)

        slab = np.zeros((2, 128, 256), np.float32)
        negmask = np.zeros((2, 256), np.float32)
        for j in range(2):
            m = 2 * i + j
            c0 = 128 * (m - 1)
            clo = max(c0, 0)
            slab[j, :, clo - c0:] = pos_bias[128 * m:128 * (m + 1),
                                             clo:c0 + 256]
            if m == 0:
                negmask[j, :128] = -1e30
        carrymask = np.zeros((2, 2, 32, 128), np.float32)
        for j in range(2):
            m = 2 * i + j
            for kind in range(2):
                for r_ in range(32):
                    beta = 2 * (r_ // 4) + (r_ % 4) // 2
                    if r_ % 2 == kind and beta <= m - 2:
                        carrymask[j, kind, r_, :] = 1.0
        in_maps.append({
            "key_s": key_s, "value_s": val_s, "query_s": qry_s,
            "wT": wT, "biases": biases, "consts": consts,
            "slab": np.ascontiguousarray(slab), "negmask": negmask,
            "carrymask": carrymask,
        })
    return in_maps


def kernel(**inputs):
    from concourse.bass_utils import run_bass_kernel_spmd
    if "nc" not in _CACHE:
        _CACHE["nc"] = _build()
    nc = _CACHE["nc"]
    inputs = {k: np.asarray(v, dtype=np.float32) for k, v in inputs.items()}
    in_maps = _host_inputs(**inputs)
    res = run_bass_kernel_spmd(nc, in_maps, core_ids=list(range(NCORES)),
                               trace=False)
    out = np.concatenate([res.results[i]["out"] for i in range(NCORES)],
                         axis=0)
    return out.astype(np.float32)
